# revision 7
# baseline (speedup 1.0000x reference)
"""Embedding gather-sum kernel for Trainium2 (8 NeuronCores, SPMD).

Problem: out[b,s,:] = sum_k W[:, ids[b,s,k]] + bias
  B=1024, S=50, K=20, E=64, V=100000 (f32 weights, int ids).

Sharding: data-parallel over batch — each of the 8 cores handles 128 batch
rows (6400 tokens, 128000 lookups).  W.T ([V, 64] f32) is replicated per
core in DRAM.

Per core, per chunk of 640 tokens (= 12800 lookup slots):
  1. ids are split host-side into 4 vocab ranges of 32768 rows (range =
     id >> 15, local = id & 32767 -> int16-safe) as compact, padded index
     lists.
  2. dma_gather fetches each range's rows from W.T into a compact SBUF tile.
  3. dma_scatter_add (SBUF parity-split dest) routes every row to its unique
     (token, k) slot.  Slots are unique, so the CCE read-modify-write never
     races.
  4. VectorE sums the 20 k-slots per token (+bias) and the result is DMA'd
     out.
Each range uses its own SWDGE queue so the descriptor rings ping-pong
between one confirmed and one new entry.
"""

import numpy as np

import concourse.bass as bass
import concourse.tile as tile
from concourse import bacc, mybir
from concourse.bass_utils import run_bass_kernel_spmd

B, S, K, E, V = 1024, 50, 20, 64, 100000
N_CORES = 8
P = 128
TOK_CORE = B // N_CORES * S          # 6400 tokens per core
T_CHUNK = 640                        # tokens per chunk
CH = TOK_CORE // T_CHUNK             # 10 chunks
TSUB = T_CHUNK // P                  # 5 token-rows per partition per chunk
SLOTS = T_CHUNK * K                  # 12800 (t,k) slots per chunk
NGRP = SLOTS // P // 2               # 50 parity groups
DUMMY_SLOT = SLOTS // P              # slot# 100 -> group 50 (garbage column)

RANGE_BASES = [0, 32768, 65536, 98304]
RANGE_SIZES = [32768, 32768, 32768, V - 98304]
# static padded list lengths per range (multiple of 128); binomial mean for
# ranges 0-2 is 12800*32768/100000 = 4194 (sigma ~53), range 3 mean 217
N_SLOTS_DEFAULT = (4608, 4608, 4608, 384)

DMA_SCRATCH = 32768

_cache: dict = {}


def _build(n_slots=N_SLOTS_DEFAULT, n_repeat=1, bufs_cg=1, bufs_op=2):
    nw16 = sum(n_slots) // 16        # idx columns (int16) per chunk
    nc = bacc.Bacc("TRN2", target_bir_lowering=False, debug=False,
                   num_devices=N_CORES,
                   dynamic_dma_scratch_size=DMA_SCRATCH)
    wt = nc.dram_tensor("wt", [V, E], mybir.dt.float32, kind="ExternalInput")
    gidx = nc.dram_tensor("gidx", [CH, P, nw16], mybir.dt.int16,
                          kind="ExternalInput")
    sidx = nc.dram_tensor("sidx", [CH, P, nw16], mybir.dt.int16,
                          kind="ExternalInput")
    bias = nc.dram_tensor("bias", [P, TSUB, E], mybir.dt.float32,
                          kind="ExternalInput")
    y = nc.dram_tensor("y", [CH, TSUB, P, E], mybir.dt.float32,
                       kind="ExternalOutput")

    with tile.TileContext(nc) as tc:
        with (
            tc.tile_pool(name="idxp", bufs=2) as idxp,
            tc.tile_pool(name="biasp", bufs=1) as biasp,
            tc.tile_pool(name="cgp", bufs=bufs_cg) as cgp,
            tc.tile_pool(name="opp", bufs=bufs_op) as opp,
            tc.tile_pool(name="accp", bufs=2) as accp,
        ):
            bias_t = biasp.tile([P, TSUB, E], mybir.dt.float32)
            nc.sync.dma_start(out=bias_t[:, :, :], in_=bias[:, :, :])

            for _ in range(n_repeat):
                for c in range(CH):
                    gidx_t = idxp.tile([P, nw16], mybir.dt.int16, tag="gidx")
                    nc.sync.dma_start(out=gidx_t[:, :], in_=gidx[c])
                    sidx_t = idxp.tile([P, nw16], mybir.dt.int16, tag="sidx")
                    nc.sync.dma_start(out=sidx_t[:, :], in_=sidx[c])

                    own = opp.tile([P, NGRP + 1, E], mybir.dt.float32, tag="own")
                    peer = opp.tile([P, NGRP + 1, E], mybir.dt.float32, tag="peer")
                    nc.vector.memset(own[:, :, :], 0.0)
                    nc.vector.memset(peer[:, :, :], 0.0)

                    off = 0
                    for r in range(4):
                        n_r = n_slots[r]
                        cg = cgp.tile([P, n_r // P, E], mybir.dt.float32,
                                      tag=f"cg{r}")
                        nc.gpsimd.dma_gather(
                            out_ap=cg[:, :, :],
                            in_ap=wt[RANGE_BASES[r]:
                                     RANGE_BASES[r] + RANGE_SIZES[r], :],
                            idxs_ap=gidx_t[:, off:off + n_r // 16],
                            num_idxs=n_r,
                            num_idxs_reg=n_r,
                            elem_size=E,
                            single_packet=False,
                        )
                        nc.gpsimd.dma_scatter_add(
                            out_ap=own[:, :, :],
                            in_ap=cg[:, :, :],
                            idxs_ap=sidx_t[:, off:off + n_r // 16],
                            num_idxs=n_r,
                            num_idxs_reg=n_r,
                            elem_size=E,
                            sbuf_tokens_per_rank=P,
                            parity_reg=0,
                            out_ap_other=peer[:, :, :],
                            single_packet=False,
                        )
                        off += n_r // 16

                    # own[p, sub*K/2 + j, :] holds k=2j, peer holds k=2j+1
                    own4 = own[:, 0:NGRP, :].rearrange(
                        "p (sub j) e -> p sub j e", j=K // 2)
                    peer4 = peer[:, 0:NGRP, :].rearrange(
                        "p (sub j) e -> p sub j e", j=K // 2)
                    acc = accp.tile([P, TSUB, E], mybir.dt.float32, tag="acc")
                    nc.vector.tensor_tensor(
                        out=acc[:, :, :], in0=own4[:, :, 0, :],
                        in1=bias_t[:, :, :], op=mybir.AluOpType.add)
                    for j in range(1, K // 2):
                        nc.vector.tensor_tensor(
                            out=acc[:, :, :], in0=own4[:, :, j, :],
                            in1=acc[:, :, :], op=mybir.AluOpType.add)
                    for j in range(K // 2):
                        nc.vector.tensor_tensor(
                            out=acc[:, :, :], in0=peer4[:, :, j, :],
                            in1=acc[:, :, :], op=mybir.AluOpType.add)
                    nc.sync.dma_start(
                        out=y[c].transpose([1, 0, 2]), in_=acc[:, :, :])
    nc.compile()
    return nc


def _wrap16(flat: np.ndarray) -> np.ndarray:
    """int16 list -> [128, n/16] layout (i at partition i%16, col i//16,
    replicated across the 8 16-partition groups)."""
    n = flat.shape[0]
    blk = flat.reshape(n // 16, 16).T            # [16, n/16]
    return np.tile(blk, (8, 1))


def _build_indices(ids_core: np.ndarray, n_slots) -> tuple[np.ndarray, np.ndarray]:
    """ids_core: [TOK_CORE, K] int32 -> (gidx, sidx) each [CH, P, nw16] int16."""
    nw16 = sum(n_slots) // 16
    gidx = np.zeros((CH, P, nw16), np.int16)
    sidx = np.zeros((CH, P, nw16), np.int16)
    t = np.arange(SLOTS) // K
    k = np.arange(SLOTS) % K
    slot = (t % P) + P * ((t // P) * K + k)                     # scatter slot id
    pad_slot = (np.arange(max(n_slots)) % P) + P * DUMMY_SLOT
    for c in range(CH):
        ids = ids_core[c * T_CHUNK:(c + 1) * T_CHUNK]           # [T_CHUNK, K]
        flat = ids.reshape(-1)                                  # (t,k) major
        rng_id = flat >> 15
        local = flat & 32767
        off = 0
        for r in range(4):
            sel = np.nonzero(rng_id == r)[0]
            n_r = n_slots[r]
            if sel.shape[0] > n_r:
                raise OverflowError(f"range {r}: {sel.shape[0]} > {n_r}")
            g = np.zeros(n_r, np.int16)
            s = pad_slot[:n_r].astype(np.int16)
            g[:sel.shape[0]] = local[sel]
            s[:sel.shape[0]] = slot[sel]
            gidx[c, :, off:off + n_r // 16] = _wrap16(g)
            sidx[c, :, off:off + n_r // 16] = _wrap16(s)
            off += n_r // 16
    return gidx, sidx


def kernel(content_input: np.ndarray, W: np.ndarray, b: np.ndarray) -> np.ndarray:
    ids = np.ascontiguousarray(content_input).astype(np.int32).reshape(B * S, K)
    wt = np.ascontiguousarray(W.T.astype(np.float32))
    bias = np.ascontiguousarray(
        np.broadcast_to(b.astype(np.float32), (P, TSUB, E)))

    # static list sizes; grow (recompile) only if an input distribution
    # overflows the default padding
    per_core = [ids[i * TOK_CORE:(i + 1) * TOK_CORE] for i in range(N_CORES)]
    counts = np.zeros(4, np.int64)
    for pc in per_core:
        for c in range(CH):
            r = pc[c * T_CHUNK:(c + 1) * T_CHUNK].reshape(-1) >> 15
            counts = np.maximum(counts, np.bincount(r, minlength=4))
    n_slots = tuple(
        int(max(d, -(-int(cnt) // 128) * 128))
        for d, cnt in zip(N_SLOTS_DEFAULT, counts))

    key = ("nc", n_slots)
    if key not in _cache:
        _cache[key] = _build(n_slots)
    nc = _cache[key]

    in_maps = []
    for i in range(N_CORES):
        gidx, sidx = _build_indices(per_core[i], n_slots)
        in_maps.append({"wt": wt, "gidx": gidx, "sidx": sidx, "bias": bias})
    res = run_bass_kernel_spmd(nc, in_maps, core_ids=list(range(N_CORES)))
    # y[c, sub, p, :] = token (c*T_CHUNK + sub*P + p)
    out = np.concatenate(
        [res.results[i]["y"].reshape(TOK_CORE, E) for i in range(N_CORES)],
        axis=0)
    return out.reshape(B, S, E)


# revision 11
# speedup vs baseline: 1.8143x; 1.8143x over previous
"""Embedding gather-sum kernel for Trainium2 (8 NeuronCores, SPMD).

Problem: out[b,s,:] = sum_k W[:, ids[b,s,k]] + bias
  B=1024, S=50, K=20, E=64, V=100000 (f32 weights, int ids).

Sharding: data-parallel over batch — each of the 8 cores handles 128 batch
rows (6400 tokens, 128000 lookups).  W.T ([V, 64] f32) is replicated per
core in DRAM.

Per core, per chunk of 640 tokens (= 12800 lookup slots):
  1. ids are split host-side into 4 vocab ranges of 32768 rows (range =
     id >> 15, local = id & 32767 -> int16-safe) as compact, padded index
     lists.
  2. dma_gather fetches each range's rows from W.T into a compact SBUF tile.
  3. dma_scatter_add (SBUF parity-split dest) routes every row to its unique
     (token, k) slot.  Slots are unique, so the CCE read-modify-write never
     races.
  4. VectorE sums the 20 k-slots per token (+bias) and the result is DMA'd
     out.
Each range uses its own SWDGE queue so the descriptor rings ping-pong
between one confirmed and one new entry.
"""

import numpy as np

import concourse.bass as bass
import concourse.tile as tile
from concourse import bacc, mybir
from concourse.bass_utils import run_bass_kernel_spmd

B, S, K, E, V = 1024, 50, 20, 64, 100000
N_CORES = 8
P = 128
TOK_CORE = B // N_CORES * S          # 6400 tokens per core
T_CHUNK = 640                        # tokens per chunk
CH = TOK_CORE // T_CHUNK             # 10 chunks
TSUB = T_CHUNK // P                  # 5 token-rows per partition per chunk
SLOTS = T_CHUNK * K                  # 12800 (t,k) slots per chunk
NGRP = SLOTS // P // 2               # 50 parity groups
DUMMY_SLOT = SLOTS // P              # slot# 100 -> group 50 (garbage column)

RANGE_BASES = [0, 32768, 65536, 98304]
RANGE_SIZES = [32768, 32768, 32768, V - 98304]
# static padded list lengths per range (multiple of 128); binomial mean for
# ranges 0-2 is 12800*32768/100000 = 4194 (sigma ~53), range 3 mean 217
N_SLOTS_DEFAULT = (4608, 4608, 4608, 384)

DMA_SCRATCH = 32768

_cache: dict = {}


def _build(n_slots=N_SLOTS_DEFAULT, n_repeat=1, bufs_cg=1, bufs_op=2,
           stages="full"):
    nw16 = sum(n_slots) // 16        # idx columns (int16) per chunk
    nc = bacc.Bacc("TRN2", target_bir_lowering=False, debug=False,
                   num_devices=N_CORES,
                   dynamic_dma_scratch_size=DMA_SCRATCH)
    wt = nc.dram_tensor("wt", [V, E], mybir.dt.float32, kind="ExternalInput")
    gidx = nc.dram_tensor("gidx", [CH, P, nw16], mybir.dt.int16,
                          kind="ExternalInput")
    sidx = nc.dram_tensor("sidx", [CH, P, nw16], mybir.dt.int16,
                          kind="ExternalInput")
    bias = nc.dram_tensor("bias", [P, TSUB, E], mybir.dt.float32,
                          kind="ExternalInput")
    y = nc.dram_tensor("y", [CH, TSUB, P, E], mybir.dt.float32,
                       kind="ExternalOutput")

    with tile.TileContext(nc) as tc:
        with (
            tc.tile_pool(name="idxp", bufs=2) as idxp,
            tc.tile_pool(name="biasp", bufs=1) as biasp,
            tc.tile_pool(name="cgp", bufs=bufs_cg) as cgp,
            tc.tile_pool(name="opp", bufs=bufs_op) as opp,
            tc.tile_pool(name="accp", bufs=2) as accp,
        ):
            bias_t = biasp.tile([P, TSUB, E], mybir.dt.float32)
            nc.sync.dma_start(out=bias_t[:, :, :], in_=bias[:, :, :])

            for _ in range(n_repeat):
                for c in range(CH):
                    gidx_t = idxp.tile([P, nw16], mybir.dt.int16, tag="gidx")
                    nc.sync.dma_start(out=gidx_t[:, :], in_=gidx[c])
                    sidx_t = idxp.tile([P, nw16], mybir.dt.int16, tag="sidx")
                    nc.sync.dma_start(out=sidx_t[:, :], in_=sidx[c])

                    own = opp.tile([P, NGRP + 1, E], mybir.dt.float32, tag="own")
                    peer = opp.tile([P, NGRP + 1, E], mybir.dt.float32, tag="peer")
                    if stages == "full":
                        nc.vector.memset(own[:, :, :], 0.0)
                        nc.vector.memset(peer[:, :, :], 0.0)

                    off = 0
                    for r in range(4):
                        n_r = n_slots[r]
                        cg = cgp.tile([P, n_r // P, E], mybir.dt.float32,
                                      tag=f"cg{r}")
                        nc.gpsimd.dma_gather(
                            out_ap=cg[:, :, :],
                            in_ap=wt[RANGE_BASES[r]:
                                     RANGE_BASES[r] + RANGE_SIZES[r], :],
                            idxs_ap=gidx_t[:, off:off + n_r // 16],
                            num_idxs=n_r,
                            num_idxs_reg=n_r,
                            elem_size=E,
                            single_packet=False,
                        )
                        if stages != "gather":
                            nc.gpsimd.dma_scatter_add(
                                out_ap=own[:, :, :],
                                in_ap=cg[:, :, :],
                                idxs_ap=sidx_t[:, off:off + n_r // 16],
                                num_idxs=n_r,
                                num_idxs_reg=n_r,
                                elem_size=E,
                                sbuf_tokens_per_rank=P,
                                parity_reg=0,
                                out_ap_other=peer[:, :, :],
                                single_packet=False,
                            )
                        off += n_r // 16

                    if stages != "full":
                        continue
                    # own[p, sub*K/2 + j, :] holds k=2j, peer holds k=2j+1
                    own4 = own[:, 0:NGRP, :].rearrange(
                        "p (sub j) e -> p sub j e", j=K // 2)
                    peer4 = peer[:, 0:NGRP, :].rearrange(
                        "p (sub j) e -> p sub j e", j=K // 2)
                    acc = accp.tile([P, TSUB, E], mybir.dt.float32, tag="acc")
                    nc.vector.tensor_tensor(
                        out=acc[:, :, :], in0=own4[:, :, 0, :],
                        in1=bias_t[:, :, :], op=mybir.AluOpType.add)
                    for j in range(1, K // 2):
                        nc.vector.tensor_tensor(
                            out=acc[:, :, :], in0=own4[:, :, j, :],
                            in1=acc[:, :, :], op=mybir.AluOpType.add)
                    for j in range(K // 2):
                        nc.vector.tensor_tensor(
                            out=acc[:, :, :], in0=peer4[:, :, j, :],
                            in1=acc[:, :, :], op=mybir.AluOpType.add)
                    nc.sync.dma_start(
                        out=y[c].transpose([1, 0, 2]), in_=acc[:, :, :])
    nc.compile()
    return nc


def _wrap16(flat: np.ndarray) -> np.ndarray:
    """int16 list -> [128, n/16] layout (i at partition i%16, col i//16,
    replicated across the 8 16-partition groups)."""
    n = flat.shape[0]
    blk = flat.reshape(n // 16, 16).T            # [16, n/16]
    return np.tile(blk, (8, 1))


def _build_indices(ids_core: np.ndarray, n_slots) -> tuple[np.ndarray, np.ndarray]:
    """ids_core: [TOK_CORE, K] int32 -> (gidx, sidx) each [CH, P, nw16] int16."""
    nw16 = sum(n_slots) // 16
    gidx = np.zeros((CH, P, nw16), np.int16)
    sidx = np.zeros((CH, P, nw16), np.int16)
    t = np.arange(SLOTS) // K
    k = np.arange(SLOTS) % K
    slot = (t % P) + P * ((t // P) * K + k)                     # scatter slot id
    pad_slot = (np.arange(max(n_slots)) % P) + P * DUMMY_SLOT
    for c in range(CH):
        ids = ids_core[c * T_CHUNK:(c + 1) * T_CHUNK]           # [T_CHUNK, K]
        flat = ids.reshape(-1)                                  # (t,k) major
        rng_id = flat >> 15
        local = flat & 32767
        off = 0
        for r in range(4):
            sel = np.nonzero(rng_id == r)[0]
            n_r = n_slots[r]
            if sel.shape[0] > n_r:
                raise OverflowError(f"range {r}: {sel.shape[0]} > {n_r}")
            g = np.zeros(n_r, np.int16)
            s = pad_slot[:n_r].astype(np.int16)
            g[:sel.shape[0]] = local[sel]
            s[:sel.shape[0]] = slot[sel]
            gidx[c, :, off:off + n_r // 16] = _wrap16(g)
            sidx[c, :, off:off + n_r // 16] = _wrap16(s)
            off += n_r // 16
    return gidx, sidx


def kernel(content_input: np.ndarray, W: np.ndarray, b: np.ndarray) -> np.ndarray:
    ids = np.ascontiguousarray(content_input).astype(np.int32).reshape(B * S, K)
    wt = np.ascontiguousarray(W.T.astype(np.float32))
    bias = np.ascontiguousarray(
        np.broadcast_to(b.astype(np.float32), (P, TSUB, E)))

    # static list sizes; grow (recompile) only if an input distribution
    # overflows the default padding
    per_core = [ids[i * TOK_CORE:(i + 1) * TOK_CORE] for i in range(N_CORES)]
    counts = np.zeros(4, np.int64)
    for pc in per_core:
        for c in range(CH):
            r = pc[c * T_CHUNK:(c + 1) * T_CHUNK].reshape(-1) >> 15
            counts = np.maximum(counts, np.bincount(r, minlength=4))
    n_slots = tuple(
        int(max(d, -(-int(cnt) // 128) * 128))
        for d, cnt in zip(N_SLOTS_DEFAULT, counts))

    key = ("nc", n_slots)
    if key not in _cache:
        _cache[key] = _build(n_slots)
    nc = _cache[key]

    in_maps = []
    for i in range(N_CORES):
        gidx, sidx = _build_indices(per_core[i], n_slots)
        in_maps.append({"wt": wt, "gidx": gidx, "sidx": sidx, "bias": bias})
    res = run_bass_kernel_spmd(nc, in_maps, core_ids=list(range(N_CORES)))
    # y[c, sub, p, :] = token (c*T_CHUNK + sub*P + p)
    out = np.concatenate(
        [res.results[i]["y"].reshape(TOK_CORE, E) for i in range(N_CORES)],
        axis=0)
    return out.reshape(B, S, E)


# revision 14
# speedup vs baseline: 1.9591x; 1.0798x over previous
"""Embedding gather-sum kernel for Trainium2 (8 NeuronCores, SPMD).

Problem: out[b,s,:] = sum_k W[:, ids[b,s,k]] + bias
  B=1024, S=50, K=20, E=64, V=100000 (f32 weights, int ids).

Sharding: data-parallel over batch — each of the 8 cores handles 128 batch
rows (6400 tokens, 128000 lookups).  W.T ([V, 64] f32) is replicated per
core in DRAM.

Per core, per chunk of 640 tokens (= 12800 lookup slots):
  1. ids are split host-side into 4 vocab ranges of 32768 rows (range =
     id >> 15, local = id & 32767 -> int16-safe) as compact, padded index
     lists.
  2. dma_gather fetches each range's rows from W.T into a compact SBUF tile.
  3. dma_scatter_add (SBUF parity-split dest) routes every row to its unique
     (token, k) slot.  Slots are unique, so the CCE read-modify-write never
     races.
  4. VectorE sums the 20 k-slots per token (+bias) and the result is DMA'd
     out.
Each range uses its own SWDGE queue so the descriptor rings ping-pong
between one confirmed and one new entry.
"""

import numpy as np

import concourse.bass as bass
import concourse.tile as tile
from concourse import bacc, mybir
from concourse.bass_utils import run_bass_kernel_spmd

B, S, K, E, V = 1024, 50, 20, 64, 100000
N_CORES = 8
P = 128
TOK_CORE = B // N_CORES * S          # 6400 tokens per core
T_CHUNK = 640                        # tokens per chunk
CH = TOK_CORE // T_CHUNK             # 10 chunks
TSUB = T_CHUNK // P                  # 5 token-rows per partition per chunk
SLOTS = T_CHUNK * K                  # 12800 (t,k) slots per chunk
NGRP = SLOTS // P // 2               # 50 parity groups
DUMMY_SLOT = SLOTS // P              # slot# 100 -> group 50 (garbage column)

RANGE_BASES = [0, 32768, 65536, 98304]
RANGE_SIZES = [32768, 32768, 32768, V - 98304]
# static padded list lengths per range (multiple of 128); binomial mean for
# ranges 0-2 is 12800*32768/100000 = 4194 (sigma ~53), range 3 mean 217
N_SLOTS_DEFAULT = (4608, 4608, 4608, 384)

DMA_SCRATCH = 32768

_cache: dict = {}


def _build_v2(n_slots=N_SLOTS_DEFAULT, n_repeat=1, bufs_cg=1, bufs_op=2,
           stages="full"):
    nw16 = sum(n_slots) // 16        # idx columns (int16) per chunk
    nc = bacc.Bacc("TRN2", target_bir_lowering=False, debug=False,
                   num_devices=N_CORES,
                   dynamic_dma_scratch_size=DMA_SCRATCH)
    wt = nc.dram_tensor("wt", [V, E], mybir.dt.float32, kind="ExternalInput")
    gidx = nc.dram_tensor("gidx", [CH, P, nw16], mybir.dt.int16,
                          kind="ExternalInput")
    sidx = nc.dram_tensor("sidx", [CH, P, nw16], mybir.dt.int16,
                          kind="ExternalInput")
    bias = nc.dram_tensor("bias", [P, TSUB, E], mybir.dt.float32,
                          kind="ExternalInput")
    y = nc.dram_tensor("y", [CH, TSUB, P, E], mybir.dt.float32,
                       kind="ExternalOutput")

    with tile.TileContext(nc) as tc:
        with (
            tc.tile_pool(name="idxp", bufs=2) as idxp,
            tc.tile_pool(name="biasp", bufs=1) as biasp,
            tc.tile_pool(name="cgp", bufs=bufs_cg) as cgp,
            tc.tile_pool(name="opp", bufs=bufs_op) as opp,
            tc.tile_pool(name="accp", bufs=2) as accp,
        ):
            bias_t = biasp.tile([P, TSUB, E], mybir.dt.float32)
            nc.sync.dma_start(out=bias_t[:, :, :], in_=bias[:, :, :])

            for _ in range(n_repeat):
                for c in range(CH):
                    gidx_t = idxp.tile([P, nw16], mybir.dt.int16, tag="gidx")
                    nc.sync.dma_start(out=gidx_t[:, :], in_=gidx[c])
                    sidx_t = idxp.tile([P, nw16], mybir.dt.int16, tag="sidx")
                    nc.sync.dma_start(out=sidx_t[:, :], in_=sidx[c])

                    own = opp.tile([P, NGRP + 1, E], mybir.dt.float32, tag="own")
                    peer = opp.tile([P, NGRP + 1, E], mybir.dt.float32, tag="peer")
                    if stages == "full":
                        nc.vector.memset(own[:, :, :], 0.0)
                        nc.vector.memset(peer[:, :, :], 0.0)

                    off = 0
                    for r in range(4):
                        n_r = n_slots[r]
                        cg = cgp.tile([P, n_r // P, E], mybir.dt.float32,
                                      tag=f"cg{r}")
                        nc.gpsimd.dma_gather(
                            out_ap=cg[:, :, :],
                            in_ap=wt[RANGE_BASES[r]:
                                     RANGE_BASES[r] + RANGE_SIZES[r], :],
                            idxs_ap=gidx_t[:, off:off + n_r // 16],
                            num_idxs=n_r,
                            num_idxs_reg=n_r,
                            elem_size=E,
                            single_packet=False,
                        )
                        if stages != "gather":
                            nc.gpsimd.dma_scatter_add(
                                out_ap=own[:, :, :],
                                in_ap=cg[:, :, :],
                                idxs_ap=sidx_t[:, off:off + n_r // 16],
                                num_idxs=n_r,
                                num_idxs_reg=n_r,
                                elem_size=E,
                                sbuf_tokens_per_rank=P,
                                parity_reg=0,
                                out_ap_other=peer[:, :, :],
                                single_packet=False,
                            )
                        off += n_r // 16

                    if stages != "full":
                        continue
                    # own[p, sub*K/2 + j, :] holds k=2j, peer holds k=2j+1
                    own4 = own[:, 0:NGRP, :].rearrange(
                        "p (sub j) e -> p sub j e", j=K // 2)
                    peer4 = peer[:, 0:NGRP, :].rearrange(
                        "p (sub j) e -> p sub j e", j=K // 2)
                    acc = accp.tile([P, TSUB, E], mybir.dt.float32, tag="acc")
                    nc.vector.tensor_tensor(
                        out=acc[:, :, :], in0=own4[:, :, 0, :],
                        in1=bias_t[:, :, :], op=mybir.AluOpType.add)
                    for j in range(1, K // 2):
                        nc.vector.tensor_tensor(
                            out=acc[:, :, :], in0=own4[:, :, j, :],
                            in1=acc[:, :, :], op=mybir.AluOpType.add)
                    for j in range(K // 2):
                        nc.vector.tensor_tensor(
                            out=acc[:, :, :], in0=peer4[:, :, j, :],
                            in1=acc[:, :, :], op=mybir.AluOpType.add)
                    nc.sync.dma_start(
                        out=y[c].transpose([1, 0, 2]), in_=acc[:, :, :])
    nc.compile()
    return nc


def _wrap16(flat: np.ndarray) -> np.ndarray:
    """int16 list -> [128, n/16] layout (i at partition i%16, col i//16,
    replicated across the 8 16-partition groups)."""
    n = flat.shape[0]
    blk = flat.reshape(n // 16, 16).T            # [16, n/16]
    return np.tile(blk, (8, 1))


def _build_indices_v2(ids_core: np.ndarray, n_slots) -> tuple[np.ndarray, np.ndarray]:
    """ids_core: [TOK_CORE, K] int32 -> (gidx, sidx) each [CH, P, nw16] int16."""
    nw16 = sum(n_slots) // 16
    gidx = np.zeros((CH, P, nw16), np.int16)
    sidx = np.zeros((CH, P, nw16), np.int16)
    t = np.arange(SLOTS) // K
    k = np.arange(SLOTS) % K
    slot = (t % P) + P * ((t // P) * K + k)                     # scatter slot id
    pad_slot = (np.arange(max(n_slots)) % P) + P * DUMMY_SLOT
    for c in range(CH):
        ids = ids_core[c * T_CHUNK:(c + 1) * T_CHUNK]           # [T_CHUNK, K]
        flat = ids.reshape(-1)                                  # (t,k) major
        rng_id = flat >> 15
        local = flat & 32767
        off = 0
        for r in range(4):
            sel = np.nonzero(rng_id == r)[0]
            n_r = n_slots[r]
            if sel.shape[0] > n_r:
                raise OverflowError(f"range {r}: {sel.shape[0]} > {n_r}")
            g = np.zeros(n_r, np.int16)
            s = pad_slot[:n_r].astype(np.int16)
            g[:sel.shape[0]] = local[sel]
            s[:sel.shape[0]] = slot[sel]
            gidx[c, :, off:off + n_r // 16] = _wrap16(g)
            sidx[c, :, off:off + n_r // 16] = _wrap16(s)
            off += n_r // 16
    return gidx, sidx




# ---------------------------------------------------------------------------
# v3: gather + PE segment-matmul reduce (no scatter pass).
# Per chunk of 320 tokens (6400 slots): 4 range-compact gathers (f32 rows),
# then for each 128-slot block a [128,WIN] 0/1 selection matrix S is built on
# VectorE (is_equal of the block's token-of-slot column vs an iota row) and
# PE accumulates  psum[64, WIN] += cg_block[128,64].T @ S[128,WIN].
# PSUM holds out.T for the chunk; bias is added on eviction; the host
# transposes at the end.
# ---------------------------------------------------------------------------
T3 = 320                              # tokens per chunk (= PSUM window)
CH3 = TOK_CORE // T3                  # 20 chunks
SLOTS3 = T3 * K                       # 6400 slots per chunk
N_SLOTS3 = (2304, 2304, 2304, 256)    # padded per-range list lengths
NBLK3 = tuple(n // P for n in N_SLOTS3)
NBLK3_TOT = sum(NBLK3)                # 56 blocks per chunk


def _build_v3(n_slots=N_SLOTS3, n_repeat=1):
    nblk = tuple(n // P for n in n_slots)
    nblk_tot = sum(nblk)
    nw16 = sum(n_slots) // 16
    nc = bacc.Bacc("TRN2", target_bir_lowering=False, debug=False,
                   num_devices=N_CORES,
                   dynamic_dma_scratch_size=DMA_SCRATCH)
    wt = nc.dram_tensor("wt", [V, E], mybir.dt.float32, kind="ExternalInput")
    gidx = nc.dram_tensor("gidx", [CH3, P, nw16], mybir.dt.int16,
                          kind="ExternalInput")
    tokf = nc.dram_tensor("tokf", [CH3, P, nblk_tot], mybir.dt.float32,
                          kind="ExternalInput")
    iota = nc.dram_tensor("iota", [P, T3], mybir.dt.float32,
                          kind="ExternalInput")
    biasc = nc.dram_tensor("biasc", [E, 1], mybir.dt.float32,
                           kind="ExternalInput")
    y = nc.dram_tensor("y", [CH3, E, T3], mybir.dt.float32,
                       kind="ExternalOutput")

    with tile.TileContext(nc) as tc:
        with (
            tc.tile_pool(name="idxp", bufs=2) as idxp,
            tc.tile_pool(name="constp", bufs=1) as constp,
            tc.tile_pool(name="cgp", bufs=2) as cgp,
            tc.tile_pool(name="sp", bufs=6) as sp,
            tc.tile_pool(name="psump", bufs=2, space="PSUM") as psump,
            tc.tile_pool(name="evp", bufs=2) as evp,
        ):
            iota_t = constp.tile([P, T3], mybir.dt.float32)
            nc.sync.dma_start(out=iota_t[:, :], in_=iota[:, :])
            biasc_t = constp.tile([E, 1], mybir.dt.float32)
            nc.sync.dma_start(out=biasc_t[:, :], in_=biasc[:, :])

            for _ in range(n_repeat):
                for c in range(CH3):
                    gidx_t = idxp.tile([P, nw16], mybir.dt.int16, tag="gidx")
                    nc.sync.dma_start(out=gidx_t[:, :], in_=gidx[c])
                    tokf_t = idxp.tile([P, nblk_tot], mybir.dt.float32,
                                       tag="tokf")
                    nc.sync.dma_start(out=tokf_t[:, :], in_=tokf[c])

                    cgs = []
                    off = 0
                    for r in range(4):
                        n_r = n_slots[r]
                        cg = cgp.tile([P, n_r // P, E], mybir.dt.float32,
                                      tag=f"cg{r}")
                        nc.gpsimd.dma_gather(
                            out_ap=cg[:, :, :],
                            in_ap=wt[RANGE_BASES[r]:
                                     RANGE_BASES[r] + RANGE_SIZES[r], :],
                            idxs_ap=gidx_t[:, off:off + n_r // 16],
                            num_idxs=n_r,
                            num_idxs_reg=n_r,
                            elem_size=E,
                            single_packet=False,
                        )
                        cgs.append(cg)
                        off += n_r // 16

                    psum = psump.tile([E, T3], mybir.dt.float32, tag="ps")
                    blk = 0
                    for r in range(4):
                        for b in range(nblk[r]):
                            s_t = sp.tile([P, T3], mybir.dt.float32, tag="S")
                            nc.vector.tensor_tensor(
                                out=s_t[:, :],
                                in0=tokf_t[:, blk:blk + 1].to_broadcast([P, T3]),
                                in1=iota_t[:, :],
                                op=mybir.AluOpType.is_equal)
                            nc.tensor.matmul(
                                out=psum[:, :],
                                lhsT=cgs[r][:, b, :],
                                rhs=s_t[:, :],
                                start=(blk == 0),
                                stop=(blk == nblk_tot - 1))
                            blk += 1

                    ev = evp.tile([E, T3], mybir.dt.float32, tag="ev")
                    nc.vector.tensor_tensor(
                        out=ev[:, :], in0=psum[:, :],
                        in1=biasc_t[:, 0:1].to_broadcast([E, T3]),
                        op=mybir.AluOpType.add)
                    nc.sync.dma_start(out=y[c], in_=ev[:, :])
    nc.compile()
    return nc


def _build_indices_v3(ids_core: np.ndarray, n_slots) -> tuple[np.ndarray, np.ndarray]:
    """ids_core: [TOK_CORE, K] int32 -> (gidx [CH3,P,nw16] i16,
    tokf [CH3,P,nblk_tot] f32)."""
    nblk = tuple(n // P for n in n_slots)
    nblk_tot = sum(nblk)
    nw16 = sum(n_slots) // 16
    gidx = np.zeros((CH3, P, nw16), np.int16)
    tokf = np.zeros((CH3, P, nblk_tot), np.float32)
    for c in range(CH3):
        flat = ids_core[c * T3:(c + 1) * T3].reshape(-1)      # [SLOTS3]
        rng_id = flat >> 15
        local = flat & 32767
        tok_of_slot = np.arange(SLOTS3) // K
        off = 0
        boff = 0
        for r in range(4):
            sel = np.nonzero(rng_id == r)[0]
            n_r = n_slots[r]
            if sel.shape[0] > n_r:
                raise OverflowError(f"range {r}: {sel.shape[0]} > {n_r}")
            g = np.zeros(n_r, np.int16)
            g[:sel.shape[0]] = local[sel]
            tf = np.full(n_r, -1.0, np.float32)
            tf[:sel.shape[0]] = tok_of_slot[sel]
            gidx[c, :, off:off + n_r // 16] = _wrap16(g)
            tokf[c, :, boff:boff + nblk[r]] = tf.reshape(nblk[r], P).T
            off += n_r // 16
            boff += nblk[r]
    return gidx, tokf


def kernel(content_input: np.ndarray, W: np.ndarray, b: np.ndarray) -> np.ndarray:
    ids = np.ascontiguousarray(content_input).astype(np.int32).reshape(B * S, K)
    wt = np.ascontiguousarray(W.T.astype(np.float32))
    iota = np.ascontiguousarray(
        np.broadcast_to(np.arange(T3, dtype=np.float32), (P, T3)))
    biasc = np.ascontiguousarray(b.astype(np.float32).reshape(E, 1))

    # static list sizes; grow (recompile) only if an input distribution
    # overflows the default padding
    per_core = [ids[i * TOK_CORE:(i + 1) * TOK_CORE] for i in range(N_CORES)]
    counts = np.zeros(4, np.int64)
    for pc in per_core:
        for c in range(CH3):
            r = pc[c * T3:(c + 1) * T3].reshape(-1) >> 15
            counts = np.maximum(counts, np.bincount(r, minlength=4))
    n_slots = tuple(
        int(max(d, -(-int(cnt) // P) * P))
        for d, cnt in zip(N_SLOTS3, counts))

    key = ("nc3", n_slots)
    if key not in _cache:
        _cache[key] = _build_v3(n_slots)
    nc = _cache[key]

    in_maps = []
    for i in range(N_CORES):
        gidx, tokf = _build_indices_v3(per_core[i], n_slots)
        in_maps.append({"wt": wt, "gidx": gidx, "tokf": tokf,
                        "iota": iota, "biasc": biasc})
    res = run_bass_kernel_spmd(nc, in_maps, core_ids=list(range(N_CORES)))
    # y[c, :, t] = out[token c*T3 + t, :]
    out = np.concatenate(
        [res.results[i]["y"].transpose(0, 2, 1).reshape(TOK_CORE, E)
         for i in range(N_CORES)],
        axis=0)
    return out.reshape(B, S, E)


# revision 16
# speedup vs baseline: 2.0674x; 1.0553x over previous
"""Embedding gather-sum kernel for Trainium2 (8 NeuronCores, SPMD).

Problem: out[b,s,:] = sum_k W[:, ids[b,s,k]] + bias
  B=1024, S=50, K=20, E=64, V=100000 (f32 weights, int ids).

Sharding: data-parallel over batch — each of the 8 cores handles 128 batch
rows (6400 tokens, 128000 lookups).  W.T ([V, 64] f32) is replicated per
core in DRAM.

Per core, per chunk of 640 tokens (= 12800 lookup slots):
  1. ids are split host-side into 4 vocab ranges of 32768 rows (range =
     id >> 15, local = id & 32767 -> int16-safe) as compact, padded index
     lists.
  2. dma_gather fetches each range's rows from W.T into a compact SBUF tile.
  3. dma_scatter_add (SBUF parity-split dest) routes every row to its unique
     (token, k) slot.  Slots are unique, so the CCE read-modify-write never
     races.
  4. VectorE sums the 20 k-slots per token (+bias) and the result is DMA'd
     out.
Each range uses its own SWDGE queue so the descriptor rings ping-pong
between one confirmed and one new entry.
"""

import numpy as np

import concourse.bass as bass
import concourse.tile as tile
from concourse import bacc, mybir
from concourse.bass_utils import run_bass_kernel_spmd

B, S, K, E, V = 1024, 50, 20, 64, 100000
N_CORES = 8
P = 128
TOK_CORE = B // N_CORES * S          # 6400 tokens per core
T_CHUNK = 640                        # tokens per chunk
CH = TOK_CORE // T_CHUNK             # 10 chunks
TSUB = T_CHUNK // P                  # 5 token-rows per partition per chunk
SLOTS = T_CHUNK * K                  # 12800 (t,k) slots per chunk
NGRP = SLOTS // P // 2               # 50 parity groups
DUMMY_SLOT = SLOTS // P              # slot# 100 -> group 50 (garbage column)

RANGE_BASES = [0, 32768, 65536, 98304]
RANGE_SIZES = [32768, 32768, 32768, V - 98304]
# static padded list lengths per range (multiple of 128); binomial mean for
# ranges 0-2 is 12800*32768/100000 = 4194 (sigma ~53), range 3 mean 217
N_SLOTS_DEFAULT = (4608, 4608, 4608, 384)

DMA_SCRATCH = 32768

_cache: dict = {}


def _build_v2(n_slots=N_SLOTS_DEFAULT, n_repeat=1, bufs_cg=1, bufs_op=2,
           stages="full"):
    nw16 = sum(n_slots) // 16        # idx columns (int16) per chunk
    nc = bacc.Bacc("TRN2", target_bir_lowering=False, debug=False,
                   num_devices=N_CORES,
                   dynamic_dma_scratch_size=DMA_SCRATCH)
    wt = nc.dram_tensor("wt", [V, E], mybir.dt.float32, kind="ExternalInput")
    gidx = nc.dram_tensor("gidx", [CH, P, nw16], mybir.dt.int16,
                          kind="ExternalInput")
    sidx = nc.dram_tensor("sidx", [CH, P, nw16], mybir.dt.int16,
                          kind="ExternalInput")
    bias = nc.dram_tensor("bias", [P, TSUB, E], mybir.dt.float32,
                          kind="ExternalInput")
    y = nc.dram_tensor("y", [CH, TSUB, P, E], mybir.dt.float32,
                       kind="ExternalOutput")

    with tile.TileContext(nc) as tc:
        with (
            tc.tile_pool(name="idxp", bufs=2) as idxp,
            tc.tile_pool(name="biasp", bufs=1) as biasp,
            tc.tile_pool(name="cgp", bufs=bufs_cg) as cgp,
            tc.tile_pool(name="opp", bufs=bufs_op) as opp,
            tc.tile_pool(name="accp", bufs=2) as accp,
        ):
            bias_t = biasp.tile([P, TSUB, E], mybir.dt.float32)
            nc.sync.dma_start(out=bias_t[:, :, :], in_=bias[:, :, :])

            for _ in range(n_repeat):
                for c in range(CH):
                    gidx_t = idxp.tile([P, nw16], mybir.dt.int16, tag="gidx")
                    nc.sync.dma_start(out=gidx_t[:, :], in_=gidx[c])
                    sidx_t = idxp.tile([P, nw16], mybir.dt.int16, tag="sidx")
                    nc.sync.dma_start(out=sidx_t[:, :], in_=sidx[c])

                    own = opp.tile([P, NGRP + 1, E], mybir.dt.float32, tag="own")
                    peer = opp.tile([P, NGRP + 1, E], mybir.dt.float32, tag="peer")
                    if stages == "full":
                        nc.vector.memset(own[:, :, :], 0.0)
                        nc.vector.memset(peer[:, :, :], 0.0)

                    off = 0
                    for r in range(4):
                        n_r = n_slots[r]
                        cg = cgp.tile([P, n_r // P, E], mybir.dt.float32,
                                      tag=f"cg{r}")
                        nc.gpsimd.dma_gather(
                            out_ap=cg[:, :, :],
                            in_ap=wt[RANGE_BASES[r]:
                                     RANGE_BASES[r] + RANGE_SIZES[r], :],
                            idxs_ap=gidx_t[:, off:off + n_r // 16],
                            num_idxs=n_r,
                            num_idxs_reg=n_r,
                            elem_size=E,
                            single_packet=False,
                        )
                        if stages != "gather":
                            nc.gpsimd.dma_scatter_add(
                                out_ap=own[:, :, :],
                                in_ap=cg[:, :, :],
                                idxs_ap=sidx_t[:, off:off + n_r // 16],
                                num_idxs=n_r,
                                num_idxs_reg=n_r,
                                elem_size=E,
                                sbuf_tokens_per_rank=P,
                                parity_reg=0,
                                out_ap_other=peer[:, :, :],
                                single_packet=False,
                            )
                        off += n_r // 16

                    if stages != "full":
                        continue
                    # own[p, sub*K/2 + j, :] holds k=2j, peer holds k=2j+1
                    own4 = own[:, 0:NGRP, :].rearrange(
                        "p (sub j) e -> p sub j e", j=K // 2)
                    peer4 = peer[:, 0:NGRP, :].rearrange(
                        "p (sub j) e -> p sub j e", j=K // 2)
                    acc = accp.tile([P, TSUB, E], mybir.dt.float32, tag="acc")
                    nc.vector.tensor_tensor(
                        out=acc[:, :, :], in0=own4[:, :, 0, :],
                        in1=bias_t[:, :, :], op=mybir.AluOpType.add)
                    for j in range(1, K // 2):
                        nc.vector.tensor_tensor(
                            out=acc[:, :, :], in0=own4[:, :, j, :],
                            in1=acc[:, :, :], op=mybir.AluOpType.add)
                    for j in range(K // 2):
                        nc.vector.tensor_tensor(
                            out=acc[:, :, :], in0=peer4[:, :, j, :],
                            in1=acc[:, :, :], op=mybir.AluOpType.add)
                    nc.sync.dma_start(
                        out=y[c].transpose([1, 0, 2]), in_=acc[:, :, :])
    nc.compile()
    return nc


def _wrap16(flat: np.ndarray) -> np.ndarray:
    """int16 list -> [128, n/16] layout (i at partition i%16, col i//16,
    replicated across the 8 16-partition groups)."""
    n = flat.shape[0]
    blk = flat.reshape(n // 16, 16).T            # [16, n/16]
    return np.tile(blk, (8, 1))


def _build_indices_v2(ids_core: np.ndarray, n_slots) -> tuple[np.ndarray, np.ndarray]:
    """ids_core: [TOK_CORE, K] int32 -> (gidx, sidx) each [CH, P, nw16] int16."""
    nw16 = sum(n_slots) // 16
    gidx = np.zeros((CH, P, nw16), np.int16)
    sidx = np.zeros((CH, P, nw16), np.int16)
    t = np.arange(SLOTS) // K
    k = np.arange(SLOTS) % K
    slot = (t % P) + P * ((t // P) * K + k)                     # scatter slot id
    pad_slot = (np.arange(max(n_slots)) % P) + P * DUMMY_SLOT
    for c in range(CH):
        ids = ids_core[c * T_CHUNK:(c + 1) * T_CHUNK]           # [T_CHUNK, K]
        flat = ids.reshape(-1)                                  # (t,k) major
        rng_id = flat >> 15
        local = flat & 32767
        off = 0
        for r in range(4):
            sel = np.nonzero(rng_id == r)[0]
            n_r = n_slots[r]
            if sel.shape[0] > n_r:
                raise OverflowError(f"range {r}: {sel.shape[0]} > {n_r}")
            g = np.zeros(n_r, np.int16)
            s = pad_slot[:n_r].astype(np.int16)
            g[:sel.shape[0]] = local[sel]
            s[:sel.shape[0]] = slot[sel]
            gidx[c, :, off:off + n_r // 16] = _wrap16(g)
            sidx[c, :, off:off + n_r // 16] = _wrap16(s)
            off += n_r // 16
    return gidx, sidx




# ---------------------------------------------------------------------------
# v3: gather + PE segment-matmul reduce (no scatter pass).
# Per chunk of 320 tokens (6400 slots): 4 range-compact gathers (f32 rows),
# then for each 128-slot block a [128,WIN] 0/1 selection matrix S is built on
# VectorE (is_equal of the block's token-of-slot column vs an iota row) and
# PE accumulates  psum[64, WIN] += cg_block[128,64].T @ S[128,WIN].
# PSUM holds out.T for the chunk; bias is added on eviction; the host
# transposes at the end.
# ---------------------------------------------------------------------------
T3 = 320                              # tokens per chunk (= PSUM window)
CH3 = TOK_CORE // T3                  # 20 chunks
SLOTS3 = T3 * K                       # 6400 slots per chunk
N_SLOTS3 = (2304, 2304, 2304, 256)    # padded per-range list lengths
NBLK3 = tuple(n // P for n in N_SLOTS3)
NBLK3_TOT = sum(NBLK3)                # 56 blocks per chunk


def _build_v3(n_slots=N_SLOTS3, n_repeat=1, reg_counts=None):
    nblk = tuple(n // P for n in n_slots)
    nblk_tot = sum(nblk)
    nw16 = sum(n_slots) // 16
    nc = bacc.Bacc("TRN2", target_bir_lowering=False, debug=False,
                   num_devices=N_CORES,
                   dynamic_dma_scratch_size=DMA_SCRATCH)
    wt = nc.dram_tensor("wt", [V, E], mybir.dt.float32, kind="ExternalInput")
    gidx = nc.dram_tensor("gidx", [CH3, P, nw16], mybir.dt.int16,
                          kind="ExternalInput")
    tokf = nc.dram_tensor("tokf", [CH3, P, nblk_tot], mybir.dt.float32,
                          kind="ExternalInput")
    iota = nc.dram_tensor("iota", [P, T3], mybir.dt.float32,
                          kind="ExternalInput")
    biasc = nc.dram_tensor("biasc", [E, 1], mybir.dt.float32,
                           kind="ExternalInput")
    y = nc.dram_tensor("y", [CH3, E, T3], mybir.dt.float32,
                       kind="ExternalOutput")

    with tile.TileContext(nc) as tc:
        with (
            tc.tile_pool(name="idxp", bufs=2) as idxp,
            tc.tile_pool(name="constp", bufs=1) as constp,
            tc.tile_pool(name="cgp", bufs=2) as cgp,
            tc.tile_pool(name="sp", bufs=6) as sp,
            tc.tile_pool(name="psump", bufs=2, space="PSUM") as psump,
            tc.tile_pool(name="evp", bufs=2) as evp,
        ):
            iota_t = constp.tile([P, T3], mybir.dt.float32)
            nc.sync.dma_start(out=iota_t[:, :], in_=iota[:, :])
            biasc_t = constp.tile([E, 1], mybir.dt.float32)
            nc.sync.dma_start(out=biasc_t[:, :], in_=biasc[:, :])

            for _ in range(n_repeat):
                for c in range(CH3):
                    gidx_t = idxp.tile([P, nw16], mybir.dt.int16, tag="gidx")
                    nc.sync.dma_start(out=gidx_t[:, :], in_=gidx[c])
                    tokf_t = idxp.tile([P, nblk_tot], mybir.dt.float32,
                                       tag="tokf")
                    nc.sync.dma_start(out=tokf_t[:, :], in_=tokf[c])

                    cgs = []
                    off = 0
                    for r in range(4):
                        n_r = n_slots[r]
                        cg = cgp.tile([P, n_r // P, E], mybir.dt.float32,
                                      tag=f"cg{r}")
                        n_used = (reg_counts[c][r]
                                  if reg_counts is not None else n_r)
                        nc.gpsimd.dma_gather(
                            out_ap=cg[:, :, :],
                            in_ap=wt[RANGE_BASES[r]:
                                     RANGE_BASES[r] + RANGE_SIZES[r], :],
                            idxs_ap=gidx_t[:, off:off + n_r // 16],
                            num_idxs=n_r,
                            num_idxs_reg=n_used,
                            elem_size=E,
                            single_packet=False,
                        )
                        cgs.append(cg)
                        off += n_r // 16

                    psum = psump.tile([E, T3], mybir.dt.float32, tag="ps")
                    blk = 0
                    for r in range(4):
                        for b in range(nblk[r]):
                            s_t = sp.tile([P, T3], mybir.dt.float32, tag="S")
                            nc.vector.tensor_tensor(
                                out=s_t[:, :],
                                in0=tokf_t[:, blk:blk + 1].to_broadcast([P, T3]),
                                in1=iota_t[:, :],
                                op=mybir.AluOpType.is_equal)
                            nc.tensor.matmul(
                                out=psum[:, :],
                                lhsT=cgs[r][:, b, :],
                                rhs=s_t[:, :],
                                start=(blk == 0),
                                stop=(blk == nblk_tot - 1))
                            blk += 1

                    ev = evp.tile([E, T3], mybir.dt.float32, tag="ev")
                    nc.vector.tensor_tensor(
                        out=ev[:, :], in0=psum[:, :],
                        in1=biasc_t[:, 0:1].to_broadcast([E, T3]),
                        op=mybir.AluOpType.add)
                    nc.sync.dma_start(out=y[c], in_=ev[:, :])
    nc.compile()
    return nc


def _build_indices_v3(ids_core: np.ndarray, n_slots, reg_counts=None) -> tuple[np.ndarray, np.ndarray]:
    """ids_core: [TOK_CORE, K] int32 -> (gidx [CH3,P,nw16] i16,
    tokf [CH3,P,nblk_tot] f32)."""
    nblk = tuple(n // P for n in n_slots)
    nblk_tot = sum(nblk)
    nw16 = sum(n_slots) // 16
    gidx = np.zeros((CH3, P, nw16), np.int16)
    tokf = np.zeros((CH3, P, nblk_tot), np.float32)
    for c in range(CH3):
        flat = ids_core[c * T3:(c + 1) * T3].reshape(-1)      # [SLOTS3]
        rng_id = flat >> 15
        local = flat & 32767
        tok_of_slot = np.arange(SLOTS3) // K
        off = 0
        boff = 0
        for r in range(4):
            sel = np.nonzero(rng_id == r)[0]
            n_r = n_slots[r]
            if sel.shape[0] > n_r:
                raise OverflowError(f"range {r}: {sel.shape[0]} > {n_r}")
            n_used = reg_counts[c][r] if reg_counts is not None else n_r
            g = np.full(n_r, -1, np.int16)     # tail ignored by descgen
            g[:n_used] = 0                      # idx-0 filler up to shared count
            g[:sel.shape[0]] = local[sel]
            tf = np.full(n_r, -1.0, np.float32)
            tf[:sel.shape[0]] = tok_of_slot[sel]
            gidx[c, :, off:off + n_r // 16] = _wrap16(g)
            tokf[c, :, boff:boff + nblk[r]] = tf.reshape(nblk[r], P).T
            off += n_r // 16
            boff += nblk[r]
    return gidx, tokf


def kernel(content_input: np.ndarray, W: np.ndarray, b: np.ndarray) -> np.ndarray:
    ids = np.ascontiguousarray(content_input).astype(np.int32).reshape(B * S, K)
    wt = np.ascontiguousarray(W.T.astype(np.float32))
    iota = np.ascontiguousarray(
        np.broadcast_to(np.arange(T3, dtype=np.float32), (P, T3)))
    biasc = np.ascontiguousarray(b.astype(np.float32).reshape(E, 1))

    # static list sizes; grow (recompile) only if an input distribution
    # overflows the default padding
    per_core = [ids[i * TOK_CORE:(i + 1) * TOK_CORE] for i in range(N_CORES)]
    # per-(chunk, range) max count across cores -> num_idxs_reg constants
    cnt = np.zeros((CH3, 4), np.int64)
    for pc in per_core:
        for c in range(CH3):
            r = pc[c * T3:(c + 1) * T3].reshape(-1) >> 15
            cnt[c] = np.maximum(cnt[c], np.bincount(r, minlength=4))
    n_slots = tuple(
        int(max(d, -(-int(m) // P) * P))
        for d, m in zip(N_SLOTS3, cnt.max(axis=0)))
    reg_counts = tuple(tuple(max(int(v), 16) for v in row) for row in cnt)

    key = ("nc3", n_slots, reg_counts)
    if key not in _cache:
        _cache[key] = _build_v3(n_slots, reg_counts=reg_counts)
    nc = _cache[key]

    in_maps = []
    for i in range(N_CORES):
        gidx, tokf = _build_indices_v3(per_core[i], n_slots, reg_counts)
        in_maps.append({"wt": wt, "gidx": gidx, "tokf": tokf,
                        "iota": iota, "biasc": biasc})
    res = run_bass_kernel_spmd(nc, in_maps, core_ids=list(range(N_CORES)))
    # y[c, :, t] = out[token c*T3 + t, :]
    out = np.concatenate(
        [res.results[i]["y"].transpose(0, 2, 1).reshape(TOK_CORE, E)
         for i in range(N_CORES)],
        axis=0)
    return out.reshape(B, S, E)
